# revision 1
# baseline (speedup 1.0000x reference)
"""HetConv (3x3 block-diagonal-by-residue + 1x1 elsewhere) on 8 trn2 cores.

Strategy: data-parallel over batch (4 images/core, weights replicated).
Per core: implicit-GEMM conv over a 66x66 zero-padded SBUF image with
channels permuted by residue mod 4 (done via strided DMA access patterns,
never materialized on host). Effective weight packs into 20 [128x128]
float32r matmul slots per spatial tile instead of 36 dense ones:
  - slots 9c+ti, c in {0,1}, ti in 0..8: tap (ky,kx)=divmod(ti,3), block-diag
    Wk for groups (2c, 2c+1); center tap also carries same-chunk W1 in its
    off-diagonal blocks.
  - slot 18/19: cross-chunk center-tap W1 (chunk0->oc chunk1 and reverse).
"""
import sys

sys.path.insert(0, "/opt/trn_rl_repo")

import numpy as np
import concourse.bacc as bacc
import concourse.mybir as mybir
from concourse import tile
from concourse.bass_utils import run_bass_kernel_spmd

N_CORES = 8
B, C, H, W = 32, 256, 64, 64
BP = B // N_CORES          # images per core
HP, WP = H + 2, W + 2      # padded image
NTILES = 8                 # output row-bands per image
RPT = H // NTILES          # rows per band
NFREE = RPT * W            # matmul moving free size (512)
NSLOTS = 20

_PROG = None


def _build(reps=1, packed=False):
    nc = bacc.Bacc("TRN2", target_bir_lowering=False, debug=False,
                   num_devices=N_CORES)
    f32 = mybir.dt.float32
    f32r = mybir.dt.float32r

    # x arrives host-padded to [BP, C, 66, 66] (zero border) so the whole
    # padded image DMAs as one contiguous run per partition.
    x = nc.dram_tensor("x", [BP, C, HP * WP], f32r, kind="ExternalInput").ap()
    w = nc.dram_tensor("w", [128, NSLOTS * 128], f32r, kind="ExternalInput").ap()
    out = nc.dram_tensor("out", [BP, C, H, W], f32, kind="ExternalOutput").ap()

    # channel c = 4k + g  ->  [b, g, k, ...]
    x_r = x.rearrange("b (k four) s -> b four k s", four=4)
    out_r = out.rearrange("b (k four) h w -> b four k h w", four=4)

    with tile.TileContext(nc) as tc:
        with (
            tc.tile_pool(name="wpool", bufs=1) as wpool,
            tc.tile_pool(name="xpool", bufs=2) as xpool,
            tc.tile_pool(name="opool", bufs=3) as opool,
            tc.tile_pool(name="pspool", bufs=3, space="PSUM") as pspool,
        ):
            wt = wpool.tile([128, NSLOTS * 128], f32r)
            nc.sync.dma_start(out=wt[:, :], in_=w[:, :])

            def wslot(s):
                return wt[:, s * 128:(s + 1) * 128]

            for img in [i % BP for i in range(BP * reps)]:
                xvs = []
                for cchunk in (0, 1):
                    xp = xpool.tile([128, HP * WP], f32r,
                                    tag=f"xp{cchunk}")
                    # partitions 0-63 <- residue 2c, 64-127 <- 2c+1; one
                    # fully-contiguous DMA per (img, chunk)
                    nc.gpsimd.dma_start(
                        out=xp[:, :],
                        in_=x_r[img, 2 * cchunk:2 * cchunk + 2],
                    )
                    xvs.append(xp[:, :].rearrange("p (h w) -> p h w", w=WP))

                for nt in range(NTILES):
                    def rhs(cchunk, ky, kx):
                        return xvs[cchunk][:, nt * RPT + ky: nt * RPT + ky + RPT,
                                           kx:kx + W]

                    for oc_chunk in (0, 1):
                        ps = pspool.tile([128, NFREE], f32, tag=f"ps{oc_chunk}")
                        if not packed:
                            for ti in range(9):
                                ky, kx = divmod(ti, 3)
                                nc.tensor.matmul(
                                    ps[:, :], wslot(9 * oc_chunk + ti),
                                    rhs(oc_chunk, ky, kx),
                                    start=(ti == 0), stop=False,
                                )
                        else:
                            # center tap first: full 128x128 (Wk diag + W1
                            # off-diag), start=True sets has_written everywhere
                            nc.tensor.matmul(
                                ps[:, :], wslot(9 * oc_chunk + 4),
                                rhs(oc_chunk, 1, 1), start=True, stop=False,
                            )
                            # non-center taps as row-strip pairs: each
                            # block-diag slot splits into two K=64, M=128
                            # matmuls on disjoint row strips (the slot's row
                            # halves are [W_geven | 0] and [0 | W_godd]).
                            # Adjacent row strips carry different taps, so
                            # the PE can overlap them (row tiling).
                            for t in (0, 1, 2, 3, 5, 6, 7, 8):
                                ky, kx = divmod(t, 3)
                                s = 9 * oc_chunk + t
                                r = rhs(oc_chunk, ky, kx)
                                nc.tensor.matmul(
                                    ps[:, :],
                                    wt[0:64, s * 128:(s + 1) * 128],
                                    r[0:64], start=False, stop=False,
                                    tile_position=(0, 0),
                                    skip_group_check=True,
                                )
                                nc.tensor.matmul(
                                    ps[:, :],
                                    wt[64:128, s * 128:(s + 1) * 128],
                                    r[64:128], start=False, stop=False,
                                    tile_position=(64, 0),
                                    skip_group_check=True,
                                )
                        # cross-chunk center-tap W1: slot 18 is ic-chunk0 ->
                        # oc-chunk1, slot 19 the reverse
                        nc.tensor.matmul(
                            ps[:, :], wslot(19 - oc_chunk),
                            rhs(1 - oc_chunk, 1, 1),
                            start=False, stop=True,
                        )
                        ot = opool.tile([128, NFREE], f32, tag=f"ot{oc_chunk}")
                        nc.vector.tensor_copy(ot[:, :], ps[:, :])
                        # one DMA per residue half, on separate queues (SP /
                        # ACT) so the two output streams run in parallel
                        engs = {(0, 0): nc.sync, (0, 1): nc.sync,
                                (1, 0): nc.scalar, (1, 1): nc.scalar}
                        for half in (0, 1):
                            g = 2 * oc_chunk + half
                            engs[(oc_chunk, half)].dma_start(
                                out=out_r[img, g, :, nt * RPT:(nt + 1) * RPT, :],
                                in_=ot[64 * half:64 * half + 64, :],
                            )

    nc.compile()
    return nc


def _get_prog():
    global _PROG
    if _PROG is None:
        _PROG = _build()
    return _PROG


def _prep_weights(Wk, W1):
    idx = [np.arange(g, 256, 4) for g in range(4)]
    wslabs = np.zeros((NSLOTS, 128, 128), np.float32)
    for c in (0, 1):
        gs = (2 * c, 2 * c + 1)
        for ti in range(9):
            ky, kx = divmod(ti, 3)
            s = 9 * c + ti
            for a in (0, 1):        # ic block position
                for b in (0, 1):    # oc block position
                    ga, gb = gs[a], gs[b]
                    if a == b:
                        blk = Wk[np.ix_(idx[gb], idx[ga])][:, :, ky, kx].T
                    elif ti == 4:
                        blk = W1[np.ix_(idx[gb], idx[ga])].T
                    else:
                        continue
                    wslabs[s, 64 * a:64 * a + 64, 64 * b:64 * b + 64] = blk
    for s, (ic_gs, oc_gs) in ((18, ((0, 1), (2, 3))), (19, ((2, 3), (0, 1)))):
        for a, ga in enumerate(ic_gs):
            for b, gb in enumerate(oc_gs):
                wslabs[s, 64 * a:64 * a + 64, 64 * b:64 * b + 64] = \
                    W1[np.ix_(idx[gb], idx[ga])].T
    # SBUF layout [K partition, slot*128 + m]
    return np.ascontiguousarray(
        wslabs.transpose(1, 0, 2).reshape(128, NSLOTS * 128))


def _make_in_maps(x, Wk, W1):
    w_host = _prep_weights(np.asarray(Wk, np.float32), np.asarray(W1, np.float32))
    xs = np.asarray(x, np.float32)
    xpad = np.zeros((B, C, HP, WP), np.float32)
    xpad[:, :, 1:H + 1, 1:W + 1] = xs
    xpad = xpad.reshape(B, C, HP * WP)
    return [
        {"x": np.ascontiguousarray(xpad[i * BP:(i + 1) * BP]), "w": w_host}
        for i in range(N_CORES)
    ]


def _run(x, Wk, W1, **spmd_kwargs):
    nc = _get_prog()
    in_maps = _make_in_maps(x, Wk, W1)
    res = run_bass_kernel_spmd(nc, in_maps, list(range(N_CORES)), **spmd_kwargs)
    outs = np.concatenate(
        [res.results[i]["out"] for i in range(N_CORES)], axis=0)
    return outs, res


def kernel(x, Wk, W1):
    return _run(x, Wk, W1)[0]



# revision 6
# speedup vs baseline: 1.4165x; 1.4165x over previous
"""HetConv (3x3 block-diagonal-by-residue + 1x1 elsewhere) on 8 trn2 cores.

Strategy: data-parallel over batch (4 images/core, weights replicated).
All matmuls run as fp8e4m3 DoubleRow (0.5 cyc/row, 2 virtual K-rows per
partition). Precision is recovered with a 3-term split computed in one
PSUM accumulation:
    16*W@x ~= Wq@xh + Wr@xh + Wq8@xl8
with Wq = q(16W), Wr = q(16W - Wq), Wq8 = q(Wq/8), xh = q(x),
xl8 = q(8*(x - xh)); the PSUM->SBUF copy applies the 1/16.

Per (row-band, oc-chunk) the 10 logical 128x128 weight slots (8 block-diag
tap slots + dense center + dense cross-chunk 1x1) become 15 DoubleRow
matmuls (5 per precision term: 4 tap pairs + 1 center/cross pair). Tap
pairs share one rhs AP [128, 2, N] whose pair-dim stride is the byte delta
between the two taps' windows in the 66x66 zero-padded SBUF image (window
of an R-row band is one contiguous 66*R run; the 2 junk columns per row
land in output positions that are never DMA'd out). Bands are 8x7+2x4
rows so the f32 PSUM tile (66*R) fits one 2KB bank.
"""
import sys

sys.path.insert(0, "/opt/trn_rl_repo")

import ml_dtypes
import numpy as np
import concourse.bacc as bacc
import concourse.mybir as mybir
from concourse import tile
from concourse.bass_utils import run_bass_kernel_spmd

N_CORES = 8
B, C, H, W = 32, 256, 64, 64
BP = B // N_CORES          # images per core
HP, WP = H + 2, W + 2      # padded image
RSZ = HP * WP + 4          # region size (+4 tail pad: last band window
                           # overruns the 66x66 image by 2 junk elements)
BANDS = [(0, 7), (7, 7), (14, 7), (21, 7), (28, 7), (35, 7), (42, 7),
         (49, 7), (56, 4), (60, 4)]
NSLOTS = 60                # 2 chunks x 3 precision terms x 10 slots
E4 = ml_dtypes.float8_e4m3

_PROG = None


def _build():
    nc = bacc.Bacc("TRN2", target_bir_lowering=False, debug=False,
                   num_devices=N_CORES)
    f32 = mybir.dt.float32
    f8 = mybir.dt.float8e4
    DR = mybir.MatmulPerfMode.DoubleRow

    # regions per image: [c0_xh, c1_xh, c0_xl8, c1_xl8], each a 66x66
    # zero-padded image (one channel per partition) + 4 tail-pad elements
    x = nc.dram_tensor("x", [BP, 128, 4 * RSZ], f8, kind="ExternalInput").ap()
    w = nc.dram_tensor("w", [128, NSLOTS * 128], f8, kind="ExternalInput").ap()
    out = nc.dram_tensor("out", [BP, C, H, W], f32, kind="ExternalOutput").ap()
    # channel ch = 4k + g  ->  [b, g, k, h, w]
    out_r = out.rearrange("b (k four) h w -> b four k h w", four=4)

    def pair_ap(base_ap, stride):
        """[p, n] -> [p, 2, n] with given pair-dim element stride."""
        v = base_ap.unsqueeze(1)
        apl = v.ap
        apl[1] = (stride, 2)
        v2 = v.copy()
        v2.ap = apl
        return v2

    with tile.TileContext(nc) as tc:
        with (
            tc.tile_pool(name="wpool", bufs=1) as wpool,
            tc.tile_pool(name="xpool", bufs=2) as xpool,
            tc.tile_pool(name="opool", bufs=3) as opool,
            tc.tile_pool(name="pspool", bufs=3, space="PSUM") as pspool,
        ):
            wt = wpool.tile([128, NSLOTS * 128], f8)
            nc.sync.dma_start(out=wt[:, :], in_=w[:, :])

            def lhsT(c, rep, j):
                # slot pair j (0..4) of (chunk c, precision term rep)
                off = ((c * 3 + rep) * 10 + 2 * j) * 128
                return wt[:, off:off + 256].rearrange(
                    "p (two m) -> p two m", two=2)

            for img in range(BP):
                xt = xpool.tile([128, 4 * RSZ], f8, tag="xt")
                nc.gpsimd.dma_start(out=xt[:, :2 * RSZ],
                                    in_=x[img, :, :2 * RSZ])
                nc.gpsimd.dma_start(out=xt[:, 2 * RSZ:],
                                    in_=x[img, :, 2 * RSZ:])

                for s0, rpt in BANDS:
                    nf = 66 * rpt
                    for c in (0, 1):
                        ps = pspool.tile([128, nf], f32, tag=f"ps{c}")

                        def win(region, dy, dx):
                            base = region * RSZ + (s0 + dy) * WP + dx
                            return xt[:, base:base + nf]

                        first = [True]

                        def mm(wap, rhs, stop=False):
                            nc.tensor.matmul(ps[:, :], wap, rhs,
                                             start=first[0], stop=stop,
                                             perf_mode=DR)
                            first[0] = False

                        for rep in (0, 1, 2):
                            xr = c + 2 if rep == 2 else c
                            dn = 2 if rep == 2 else 0
                            # center/cross dense pair: vrow0 is always the
                            # chunk-0 window; slot order per chunk matches
                            mm(lhsT(c, rep, 4),
                               pair_ap(win(dn, 1, 1), RSZ))
                            # tap pairs (t0,t2) (t3,t5) (t6,t8) (t1,t7):
                            # pair strides must be even (fp8 DoubleRow
                            # fetches 2-byte pairs; odd strides wedge the PE)
                            mm(lhsT(c, rep, 0), pair_ap(win(xr, 0, 0), 2))
                            mm(lhsT(c, rep, 1), pair_ap(win(xr, 1, 0), 2))
                            mm(lhsT(c, rep, 2), pair_ap(win(xr, 2, 0), 2))
                            mm(lhsT(c, rep, 3), pair_ap(win(xr, 0, 1), 132),
                               stop=(rep == 2))

                        # PSUM -> SBUF with the 1/16 scale, dropping the 2
                        # junk columns per row; alternate engines by chunk
                        ot = opool.tile([128, rpt * W], f32, tag=f"ot{c}")
                        src = ps[:, :].rearrange("p (r u) -> p r u", u=66)
                        src = src[:, :, 0:W]
                        dst = ot[:, :].rearrange("p (r w) -> p r w", w=W)
                        if c == 0:
                            nc.scalar.activation(
                                dst, src, mybir.ActivationFunctionType.Copy,
                                scale=1.0 / 16.0)
                        else:
                            nc.vector.tensor_scalar_mul(dst, src, 1.0 / 16.0)
                        # one full-width DMA per (band, chunk)
                        eng = nc.sync if c == 0 else nc.scalar
                        eng.dma_start(
                            out=out_r[img, 2 * c:2 * c + 2, :,
                                      s0:s0 + rpt, :],
                            in_=dst)

    nc.compile()
    return nc


def _get_prog():
    global _PROG
    if _PROG is None:
        _PROG = _build()
    return _PROG


def _prep_weights(Wk, W1):
    idx = [np.arange(g, 256, 4) for g in range(4)]
    # 10 f32 slots per chunk: [t0,t2,t3,t5,t6,t8,t1,t7, X, Y] (tap pairs
    # with even window strides) where (X, Y) = (center, cross) for c=0 and
    # (cross, center) for c=1 so the dense DoubleRow pair's vrow0 always
    # multiplies the chunk-0 window
    slots = np.zeros((2, 10, 128, 128), np.float32)
    for c in (0, 1):
        gs = (2 * c, 2 * c + 1)
        for si, t in enumerate((0, 2, 3, 5, 6, 8, 1, 7)):
            ky, kx = divmod(t, 3)
            for a in (0, 1):
                ga = gs[a]
                slots[c, si, 64 * a:64 * a + 64, 64 * a:64 * a + 64] = \
                    Wk[np.ix_(idx[ga], idx[ga])][:, :, ky, kx].T
        center = np.zeros((128, 128), np.float32)
        for a in (0, 1):        # ic block
            for b in (0, 1):    # oc block
                ga, gb = gs[a], gs[b]
                blk = (Wk[np.ix_(idx[gb], idx[ga])][:, :, 1, 1].T if a == b
                       else W1[np.ix_(idx[gb], idx[ga])].T)
                center[64 * a:64 * a + 64, 64 * b:64 * b + 64] = blk
        cross = np.zeros((128, 128), np.float32)  # ic chunk 1-c -> oc chunk c
        ogs = (2 * (1 - c), 2 * (1 - c) + 1)
        for a in (0, 1):
            for b in (0, 1):
                cross[64 * a:64 * a + 64, 64 * b:64 * b + 64] = \
                    W1[np.ix_(idx[gs[b]], idx[ogs[a]])].T
        slots[c, 8] = center if c == 0 else cross
        slots[c, 9] = cross if c == 0 else center

    wq = slots16 = 16.0 * slots
    wq = slots16.astype(E4)
    wr = (slots16 - wq.astype(np.float32)).astype(E4)
    wq8 = (wq.astype(np.float32) / 8.0).astype(E4)
    # [c, rep, slot, K, M] -> SBUF [K, (c rep slot M)]
    allw = np.stack([np.stack([wq[c], wr[c], wq8[c]]) for c in (0, 1)])
    return np.ascontiguousarray(
        allw.transpose(3, 0, 1, 2, 4).reshape(128, NSLOTS * 128))


def _prep_x(x):
    xs = np.asarray(x, np.float32)
    xpad = np.zeros((B, C, HP, WP), np.float32)
    xpad[:, :, 1:H + 1, 1:W + 1] = xs
    xh = xpad.astype(E4)
    xl8 = (8.0 * (xpad - xh.astype(np.float32))).astype(E4)
    idx = [np.arange(g, 256, 4) for g in range(4)]
    chunk_ch = [np.concatenate([idx[2 * c], idx[2 * c + 1]]) for c in (0, 1)]
    xq = np.zeros((B, 128, 4, RSZ), E4)
    for c in (0, 1):
        xq[:, :, c, :HP * WP] = \
            xh[:, chunk_ch[c]].reshape(B, 128, HP * WP)
        xq[:, :, c + 2, :HP * WP] = \
            xl8[:, chunk_ch[c]].reshape(B, 128, HP * WP)
    return xq.reshape(B, 128, 4 * RSZ)


def _make_in_maps(x, Wk, W1):
    w_host = _prep_weights(np.asarray(Wk, np.float32),
                           np.asarray(W1, np.float32))
    xq = _prep_x(x)
    return [
        {"x": np.ascontiguousarray(xq[i * BP:(i + 1) * BP]), "w": w_host}
        for i in range(N_CORES)
    ]


def _run(x, Wk, W1, **spmd_kwargs):
    nc = _get_prog()
    in_maps = _make_in_maps(x, Wk, W1)
    res = run_bass_kernel_spmd(nc, in_maps, list(range(N_CORES)), **spmd_kwargs)
    outs = np.concatenate(
        [res.results[i]["out"] for i in range(N_CORES)], axis=0)
    return outs, res


def kernel(x, Wk, W1):
    return _run(x, Wk, W1)[0]
